# revision 73
# baseline (speedup 1.0000x reference)
"""DualRelGCN message-passing kernel for 8 TRN2 NeuronCores.

Strategy (destination-sharded, collective-free, block-dense):
  - LayerNorm is invariant to positive per-row scaling, so LN(agg/denom) ==
    LN(agg): the denominator drops out of the computation entirely.
  - Shard edges by dst range: core c owns nodes [1250c, 1250(c+1)) and
    receives every edge whose dst falls there.  Each core computes its 1250
    output rows completely locally -> no collectives.
  - The weighted gather+segment_sum is expressed as a block matmul:
    agg[tile t] = sum_s W_ts.T @ X_s, where W_ts is the [128 src, 128 dst]
    dense block of the weighted adjacency (host-scattered from the edge
    list; ~5% nnz but dense matmul on PE beats any descriptor-generated
    gather path by a wide margin) and X_s is a [128, 256] tile of rel_embed
    (bf16, fully resident in SBUF).  PSUM accumulates over s in fp32.
  - Epilogue per dst tile (critical-path optimized): LN stats via
    E[x^2]-mu^2 so the rowsum (DVE) and sum-of-squares (ACT) passes read
    the PSUM aggregate in parallel; LN is one fused (sub,mult) DVE op;
    alpha folds into the PSUM->SBUF transpose copy (single ACT op for both
    halves); the 256-deep projection is ONE fp8 DoubleRow matmul; the
    residual add reads y straight from PSUM.
  - Two-depth software pipeline: tile t-1's stats/LN + transposes are
    emitted after tile t's block matmuls, and tile t-1's projection matmul
    is emitted in the MIDDLE of tile t+1's matmul run, so the in-order PE
    queue never stalls on the ACT copy of lnT.
  - The output is stored bf16 (halves the final store; the host upcasts
    to f32) and the last tile's residual-add + store run in halves so the
    final DMA starts earlier.
  - The device program is fully static: the edge distribution only changes
    tensor *contents*, never the instruction stream.

Measured on the 8-core trn2 fixture: 78.8-79.6us HW exec in a quiet
machine phase (81-93us when the shared TRN2 is contended; identical
binaries swing by several us between phases) vs the 81.8us staged
baseline; rel fro error 5.4e-3.  The
kernel is delivery-bound: ~17MB/core of loads at ~360GB/s pins the
first half (DMA data flow starts ~10us after NEFF start regardless of
ring/issue order -- DGE bring-up floor; head ~13.6us until X+W0 land,
tile-1 W gap ~2us), then the PE stream (~51us busy, at the
fp8-DoubleRow column-rate floor: 1 output column/cycle means any dense
blocking costs >=102.4K columns = 44.5us) and a ~8us tail (last tile's
serial LN chain + store + teardown) close it out.  Things measured NOT
to help: earlier PE start via finer chunks (p-state stutter eats the
gain), PE p-state warm-up matmuls (ramped the clock perfectly but the
kernel is delivery-bound, so the PE just waits faster), single-ring
need-order DMA for X+W0 (PE start pinned at ~13.6 either way, and W1+
falls behind X's tail), W or out stores on other rings (a ring's
sequencer processes its DMA sem-waits in order, so one late-dependency
store blocks every load queued behind it), bf16 residual, deeper W
prefetch (pulls far-future W bytes into the startup-critical window).
"""

import sys

for _p in ("/opt/trn_rl_repo",):
    if _p not in sys.path:
        sys.path.insert(0, _p)

from contextlib import ExitStack

import numpy as np
import ml_dtypes

import concourse.bacc as bacc
import concourse.mybir as mybir
from concourse.alu_op_type import AluOpType
from concourse.tile import TileContext
from concourse.bass_utils import run_bass_kernel_spmd

F32 = mybir.dt.float32
BF16 = mybir.dt.bfloat16
FP8 = mybir.dt.float8e4
AF = mybir.ActivationFunctionType

N_NODES = 10000
DIM = 256
N_CORES = 8
NODES_PER_CORE = N_NODES // N_CORES  # 1250
TILE = 128
N_TILES = -(-NODES_PER_CORE // TILE)  # 10 dst tiles per core
S_TILES = -(-N_NODES // TILE) + 1  # 79 src tiles + 1 zero pad -> even count
S_PAIRS = S_TILES // 2  # DoubleRow matmuls contract two src tiles at once
OUT_ROWS = N_TILES * TILE  # 1280
ALPHA = 0.1
LN_EPS = 1e-5
# chunk boundaries (src-tile units): small leading chunks let the first
# matmuls start before the bulk of X/W lands
X_BOUNDS = [0, 20, 40, 60, 80]
XDIM = DIM
W_BOUNDS0 = [0, 20, 40, 60, 80]  # dst tile 0 (startup critical)
W_BOUNDS1 = [0, 8, 20, 40, 60, 80]  # tile 1: PE restarts off a small chunk
W_BOUNDS = [0, 20, 40, 60, 80]  # steady-state tiles

_CACHE: dict = {}


def _build():
    nc = bacc.Bacc("TRN2", target_bir_lowering=False, debug=False,
                   num_devices=N_CORES)

    x_d = nc.dram_tensor("x", [128, S_TILES * XDIM], FP8,
                        kind="ExternalInput")
    w_d = nc.dram_tensor("wblk", [N_TILES, 128, S_TILES * TILE], FP8,
                         kind="ExternalInput")
    relsl_d = nc.dram_tensor("relslice", [OUT_ROWS, DIM], F32,
                             kind="ExternalInput")
    pwt_d = nc.dram_tensor("projwT", [128, 2 * DIM], FP8,
                           kind="ExternalInput")
    out_d = nc.dram_tensor("out", [OUT_ROWS, DIM], BF16,
                           kind="ExternalOutput")

    with TileContext(nc) as tc, ExitStack() as es:
        const_pool = es.enter_context(tc.tile_pool(name="const", bufs=1))
        wpool = es.enter_context(tc.tile_pool(name="wblk", bufs=4))
        ep_pool = es.enter_context(tc.tile_pool(name="ep", bufs=3))
        ps_agg = es.enter_context(tc.tile_pool(name="ps_agg", bufs=3,
                                               space="PSUM"))
        ps_tr = es.enter_context(tc.tile_pool(name="ps_tr", bufs=2,
                                              space="PSUM"))
        ps_y = es.enter_context(tc.tile_pool(name="ps_y", bufs=3,
                                             space="PSUM"))


        # --- constants / resident inputs ---
        iota_row = const_pool.tile([128, 128], F32, tag="iota")
        nc.gpsimd.iota(iota_row[:], [[1, 128]], base=0, channel_multiplier=0,
                       allow_small_or_imprecise_dtypes=True)
        pidx = const_pool.tile([128, 1], F32, tag="pidx")
        nc.gpsimd.iota(pidx[:], [[1, 1]], base=0, channel_multiplier=1,
                       allow_small_or_imprecise_dtypes=True)
        ident = const_pool.tile([128, 128], BF16, tag="ident")
        nc.vector.tensor_scalar(ident[:], iota_row[:], pidx[:], None,
                                AluOpType.is_equal)
        # rel_embed (fp8), fully resident; chunked load so dst-tile 0's
        # matmuls can start before the whole stream lands.  X rides the
        # scalar-engine HWDGE ring so it doesn't queue ahead of W's
        # sync-engine ring (per-engine FIFO).  Each dma_start costs its
        # sequencer ~0.67us, so the startup-critical X chunks are issued
        # first; pwt (needed only at the first projection, ~25us) after.
        x_sb = const_pool.tile([128, S_TILES, XDIM], FP8, tag="x")
        bounds = X_BOUNDS
        for i in range(len(bounds) - 1):
            lo, hi = bounds[i], bounds[i + 1]
            nc.scalar.dma_start(x_sb[:, lo:hi, :],
                                x_d[:, lo * XDIM:hi * XDIM])
        pwt_sb = const_pool.tile([128, 2, DIM], FP8, tag="pwt")
        nc.scalar.dma_start(pwt_sb[:], pwt_d[:])

        def ep_pre(t, agg_ps):
            # Stats + LN, no PE involvement.  var from E[x^2]-mu^2 so the
            # rowsum (DVE) and sum-of-squares (ACT) passes both read agg_ps
            # directly and run on different engines in parallel.
            rel_t = ep_pool.tile([128, DIM], F32, tag="rel")
            nc.scalar.dma_start(rel_t[:], relsl_d[t * 128:(t + 1) * 128, :])
            agg = ep_pool.tile([128, DIM], F32, tag="agg_sb")
            rowsum = ep_pool.tile([128, 1], F32, tag="rowsum")
            nc.vector.tensor_scalar(agg[:], agg_ps[:], 0.0, 0.0,
                                    AluOpType.add, AluOpType.add,
                                    accum_out=rowsum[:])
            sq = ep_pool.tile([128, DIM], BF16, tag="sq")
            sumsq = ep_pool.tile([128, 1], F32, tag="sumsq")
            nc.scalar.activation(sq[:], agg_ps[:], AF.Square,
                                 accum_out=sumsq[:])
            mean = ep_pool.tile([128, 1], F32, tag="mean")
            nc.scalar.mul(mean[:], rowsum[:], 1.0 / DIM)
            nmean = ep_pool.tile([128, 1], F32, tag="nmean")
            nc.vector.tensor_scalar(nmean[:], rowsum[:], -1.0 / DIM, None,
                                    AluOpType.mult)
            # bias_t = eps - mu^2
            bias_t = ep_pool.tile([128, 1], F32, tag="bias_t")
            nc.vector.tensor_scalar(bias_t[:], nmean[:], mean[:], LN_EPS,
                                    AluOpType.mult, AluOpType.add)
            std = ep_pool.tile([128, 1], F32, tag="std")
            nc.scalar.activation(std[:], sumsq[:], AF.Sqrt, bias=bias_t[:],
                                 scale=1.0 / DIM)
            rstd = ep_pool.tile([128, 1], F32, tag="rstd")
            nc.vector.reciprocal(rstd[:], std[:])
            ln = ep_pool.tile([128, DIM], BF16, tag="ln")
            nc.vector.tensor_scalar(ln[:], agg[:], mean[:], rstd[:],
                                    AluOpType.subtract, AluOpType.mult)
            return ln, rel_t

        def ep_tr(t, ln):
            # transpose ln (bf16) on the PE; both halves land in one PSUM
            # tile so a single ACT copy moves them out, folding in alpha
            tr_ps = ps_tr.tile([128, 2, 128], BF16, tag="tr")
            for k in range(2):
                nc.tensor.transpose(tr_ps[:, k, :],
                                    ln[:, k * 128:(k + 1) * 128], ident[:])
            lnT = ep_pool.tile([128, 2, 128], FP8, tag="lnT")
            nc.scalar.mul(lnT[:], tr_ps[:], ALPHA)
            return lnT

        def ep_y(t, lnT, rel_t, last=False):
            # one fp8 DoubleRow matmul does the whole 256-deep projection;
            # the residual add reads y straight from PSUM
            y_ps = ps_y.tile([128, DIM], F32, tag="y")
            nc.tensor.matmul(y_ps[:], lnT[:], pwt_sb[:],
                             start=True, stop=True,
                             perf_mode=mybir.MatmulPerfMode.DoubleRow)
            out_t = ep_pool.tile([128, DIM], BF16, tag="out")
            if last:
                # tail: stream the residual add + store in halves so the
                # final DMA starts half an add earlier
                for h in range(2):
                    sl = slice(h * 128, (h + 1) * 128)
                    nc.vector.tensor_tensor(out_t[:, sl], y_ps[:, sl],
                                            rel_t[:, sl], AluOpType.add)
                    nc.scalar.dma_start(out_d[t * 128:(t + 1) * 128, sl],
                                        out_t[:, sl])
            else:
                nc.vector.tensor_tensor(out_t[:], y_ps[:], rel_t[:],
                                        AluOpType.add)
                # scalar ring: keep the W (sync) FIFO free of stores
                nc.scalar.dma_start(out_d[t * 128:(t + 1) * 128, :],
                                    out_t[:])

        # software-pipelined at three depths: tile t-1's stats/LN chain is
        # emitted after tile t's MMs, its PE transposes in the MIDDLE of
        # tile t+1's MMs (a 2.4us window for the chain, so the in-order PE
        # queue doesn't stall on ln), and its projection in the MIDDLE of
        # tile t+2's MMs (a full iteration for the ACT copy of lnT).
        prev = None   # (t, agg_ps): awaiting stats/LN
        lnq = None    # (t, ln, rel_t): awaiting PE transpose
        ready = None  # (t, lnT, rel_t): awaiting projection + store
        for t in range(N_TILES):
            wb = (W_BOUNDS0 if t == 0
                  else W_BOUNDS1 if t == 1 else W_BOUNDS)
            w_t = wpool.tile([128, S_TILES, TILE], FP8, tag="w")
            for i in range(len(wb) - 1):
                lo, hi = wb[i], wb[i + 1]
                nc.sync.dma_start(w_t[:, lo:hi, :],
                                  w_d[t, :, lo * TILE:hi * TILE])

            agg_ps = ps_agg.tile([128, XDIM], F32, tag="agg")
            for j in range(S_PAIRS // 2):
                nc.tensor.matmul(agg_ps[:], w_t[:, 2 * j:2 * j + 2, :],
                                 x_sb[:, 2 * j:2 * j + 2, :],
                                 start=(j == 0), stop=False,
                                 perf_mode=mybir.MatmulPerfMode.DoubleRow)
            if ready is not None:
                ep_y(*ready)
                ready = None
            if lnq is not None:
                tq, lnp, relp = lnq
                ready = (tq, ep_tr(tq, lnp), relp)
                lnq = None
            if t == N_TILES - 1 and prev is not None:
                # final iteration: pull tile 8's stats/LN chain forward to
                # the MID slot so it hides under the last 20 agg matmuls
                tq, pps = prev
                prev = None
                lnp8, relp8 = ep_pre(tq, pps)
                lnq = (tq, lnp8, relp8)
            for j in range(S_PAIRS // 2, S_PAIRS):
                nc.tensor.matmul(agg_ps[:], w_t[:, 2 * j:2 * j + 2, :],
                                 x_sb[:, 2 * j:2 * j + 2, :],
                                 start=False, stop=(j == S_PAIRS - 1),
                                 perf_mode=mybir.MatmulPerfMode.DoubleRow)
            if prev is not None:
                tq, pps = prev
                ln, rel_t = ep_pre(tq, pps)
                lnq = (tq, ln, rel_t)
            prev = (t, agg_ps)
        # drain: ready = tile 7 (transposed), lnq = tile 8 (chain hidden
        # under aggs#2(9)), prev = tile 9.  Enqueue tile 9's chain FIRST so
        # it flows on DVE/ACT while the PE drains tiles 7/8.
        tp, pps = prev
        ln9, rel9 = ep_pre(tp, pps)
        ep_y(*ready)
        tq, lnp, relp = lnq
        ep_y(tq, ep_tr(tq, lnp), relp)
        ep_y(tp, ep_tr(tp, ln9), rel9, last=True)

    nc.compile()
    return nc


def _prep(rel_embed, rel_edge_index, rel_edge_weight, proj_w):
    """Host-side sharding/layout: scatter edges into dense per-(dst tile,
    src tile) weight blocks; lay out rel_embed for SBUF residency."""
    src = np.asarray(rel_edge_index[0], dtype=np.int64)
    dst = np.asarray(rel_edge_index[1], dtype=np.int64)
    w = np.asarray(rel_edge_weight, dtype=np.float32)
    rel = np.asarray(rel_embed, dtype=np.float32)
    pw = np.asarray(proj_w, dtype=np.float32)

    core = dst // NODES_PER_CORE
    drel = dst - core * NODES_PER_CORE
    t = drel // TILE
    d = drel % TILE
    s = src // TILE
    p = src % TILE
    # flat index inside one core's [N_TILES, S_TILES, 128, 128] block array
    flat = ((t * S_TILES + s) * TILE + p) * TILE + d
    blk_sz = N_TILES * S_TILES * TILE * TILE

    w_dev = np.empty((N_CORES, N_TILES, 128, S_TILES * TILE),
                     dtype=ml_dtypes.float8_e4m3)
    for c in range(N_CORES):
        m = core == c
        wc = np.bincount(flat[m], weights=w[m], minlength=blk_sz)
        wc = wc.reshape(N_TILES, S_TILES, TILE, TILE).astype(np.float32)
        # -> [t, p(src), s*128+d(dst)] so the SBUF tile is partition=src
        w_dev[c] = wc.transpose(0, 2, 1, 3).reshape(
            N_TILES, 128, S_TILES * TILE)

    rel16 = rel.astype(ml_dtypes.float8_e4m3)
    rel16_pad = np.zeros((S_TILES * TILE, XDIM), dtype=ml_dtypes.float8_e4m3)
    rel16_pad[:N_NODES, :DIM] = rel16
    x_dev = np.ascontiguousarray(
        rel16_pad.reshape(S_TILES, TILE, XDIM).transpose(1, 0, 2).reshape(
            128, S_TILES * XDIM))

    relslice = np.zeros((N_CORES, OUT_ROWS, DIM), dtype=np.float32)
    for c in range(N_CORES):
        relslice[c, :NODES_PER_CORE] = rel[c * NODES_PER_CORE:
                                           (c + 1) * NODES_PER_CORE]
    pwt = pw.T.astype(ml_dtypes.float8_e4m3)  # [f, o]
    pwt_dev = np.ascontiguousarray(
        pwt.reshape(2, 128, DIM).transpose(1, 0, 2).reshape(128, 2 * DIM))

    in_maps = []
    for c in range(N_CORES):
        in_maps.append({
            "x": x_dev,
            "wblk": w_dev[c],
            "relslice": relslice[c],
            "projwT": pwt_dev,
        })
    return in_maps


def kernel(rel_embed, rel_edge_index, rel_edge_weight, proj_w,
           _trace=False):
    in_maps = _prep(rel_embed, rel_edge_index, rel_edge_weight, proj_w)
    nc = _CACHE.get("nc")
    if nc is None:
        nc = _build()
        _CACHE["nc"] = nc
    res = run_bass_kernel_spmd(nc, in_maps, core_ids=list(range(N_CORES)),
                               trace=_trace)
    out = np.concatenate(
        [res.results[c]["out"][:NODES_PER_CORE] for c in range(N_CORES)],
        axis=0)
    if _trace:
        kernel.last_results = res
    return out.astype(np.float32)



# revision 74
# speedup vs baseline: 1.0109x; 1.0109x over previous
"""DualRelGCN message-passing kernel for 8 TRN2 NeuronCores.

Strategy (destination-sharded, collective-free, block-dense):
  - LayerNorm is invariant to positive per-row scaling, so LN(agg/denom) ==
    LN(agg): the denominator drops out of the computation entirely.
  - Shard edges by dst range: core c owns nodes [1250c, 1250(c+1)) and
    receives every edge whose dst falls there.  Each core computes its 1250
    output rows completely locally -> no collectives.
  - The weighted gather+segment_sum is expressed as a block matmul:
    agg[tile t] = sum_s W_ts.T @ X_s, where W_ts is the [128 src, 128 dst]
    dense block of the weighted adjacency (host-scattered from the edge
    list; ~5% nnz but dense matmul on PE beats any descriptor-generated
    gather path by a wide margin) and X_s is a [128, 256] tile of rel_embed
    (bf16, fully resident in SBUF).  PSUM accumulates over s in fp32.
  - Epilogue per dst tile (critical-path optimized): LN stats via
    E[x^2]-mu^2 so the rowsum (DVE) and sum-of-squares (ACT) passes read
    the PSUM aggregate in parallel; LN is one fused (sub,mult) DVE op;
    alpha folds into the PSUM->SBUF transpose copy (single ACT op for both
    halves); the 256-deep projection is ONE fp8 DoubleRow matmul; the
    residual add reads y straight from PSUM.
  - Two-depth software pipeline: tile t-1's stats/LN + transposes are
    emitted after tile t's block matmuls, and tile t-1's projection matmul
    is emitted in the MIDDLE of tile t+1's matmul run, so the in-order PE
    queue never stalls on the ACT copy of lnT.
  - The output is stored bf16 (halves the final store; the host upcasts
    to f32) and the last tile's residual-add + store run in halves so the
    final DMA starts earlier.
  - The device program is fully static: the edge distribution only changes
    tensor *contents*, never the instruction stream.

Measured on the 8-core trn2 fixture: 78.8-79.6us HW exec in a quiet
machine phase (81-93us when the shared TRN2 is contended; identical
binaries swing by several us between phases) vs the 81.8us staged
baseline; rel fro error 5.4e-3.  The
kernel is delivery-bound: ~17MB/core of loads at ~360GB/s pins the
first half (DMA data flow starts ~10us after NEFF start regardless of
ring/issue order -- DGE bring-up floor; head ~13.6us until X+W0 land,
tile-1 W gap ~2us), then the PE stream (~51us busy, at the
fp8-DoubleRow column-rate floor: 1 output column/cycle means any dense
blocking costs >=102.4K columns = 44.5us) and a ~8us tail (last tile's
serial LN chain + store + teardown) close it out.  Things measured NOT
to help: earlier PE start via finer chunks (p-state stutter eats the
gain), PE p-state warm-up matmuls (ramped the clock perfectly but the
kernel is delivery-bound, so the PE just waits faster), single-ring
need-order DMA for X+W0 (PE start pinned at ~13.6 either way, and W1+
falls behind X's tail), W or out stores on other rings (a ring's
sequencer processes its DMA sem-waits in order, so one late-dependency
store blocks every load queued behind it), bf16 residual, deeper W
prefetch (pulls far-future W bytes into the startup-critical window).
"""

import sys

for _p in ("/opt/trn_rl_repo",):
    if _p not in sys.path:
        sys.path.insert(0, _p)

from contextlib import ExitStack

import numpy as np
import ml_dtypes

import concourse.bacc as bacc
import concourse.mybir as mybir
from concourse.alu_op_type import AluOpType
from concourse.tile import TileContext
from concourse.bass_utils import run_bass_kernel_spmd

F32 = mybir.dt.float32
BF16 = mybir.dt.bfloat16
FP8 = mybir.dt.float8e4
AF = mybir.ActivationFunctionType

N_NODES = 10000
DIM = 256
N_CORES = 8
NODES_PER_CORE = N_NODES // N_CORES  # 1250
TILE = 128
N_TILES = -(-NODES_PER_CORE // TILE)  # 10 dst tiles per core
S_TILES = -(-N_NODES // TILE) + 1  # 79 src tiles + 1 zero pad -> even count
S_PAIRS = S_TILES // 2  # DoubleRow matmuls contract two src tiles at once
OUT_ROWS = N_TILES * TILE  # 1280
ALPHA = 0.1
LN_EPS = 1e-5
# chunk boundaries (src-tile units): small leading chunks let the first
# matmuls start before the bulk of X/W lands
X_BOUNDS = [0, 20, 40, 60, 80]
XDIM = DIM
W_BOUNDS0 = [0, 20, 40, 60, 80]  # dst tile 0 (startup critical)
W_BOUNDS1 = [0, 8, 20, 40, 60, 80]  # tile 1: PE restarts off a small chunk
W_BOUNDS = [0, 20, 40, 60, 80]  # steady-state tiles

_CACHE: dict = {}


def _build():
    nc = bacc.Bacc("TRN2", target_bir_lowering=False, debug=False,
                   num_devices=N_CORES)

    x_d = nc.dram_tensor("x", [128, S_TILES * XDIM], FP8,
                        kind="ExternalInput")
    w_d = nc.dram_tensor("wblk", [N_TILES, 128, S_TILES * TILE], FP8,
                         kind="ExternalInput")
    relsl_d = nc.dram_tensor("relslice", [OUT_ROWS, DIM], F32,
                             kind="ExternalInput")
    pwt_d = nc.dram_tensor("projwT", [128, 2 * DIM], FP8,
                           kind="ExternalInput")
    out_d = nc.dram_tensor("out", [OUT_ROWS, DIM], BF16,
                           kind="ExternalOutput")

    with TileContext(nc) as tc, ExitStack() as es:
        const_pool = es.enter_context(tc.tile_pool(name="const", bufs=1))
        wpool = es.enter_context(tc.tile_pool(name="wblk", bufs=4))
        ep_pool = es.enter_context(tc.tile_pool(name="ep", bufs=3))
        ps_agg = es.enter_context(tc.tile_pool(name="ps_agg", bufs=3,
                                               space="PSUM"))
        ps_tr = es.enter_context(tc.tile_pool(name="ps_tr", bufs=2,
                                              space="PSUM"))
        ps_y = es.enter_context(tc.tile_pool(name="ps_y", bufs=3,
                                             space="PSUM"))


        # --- constants / resident inputs ---
        iota_row = const_pool.tile([128, 128], F32, tag="iota")
        nc.gpsimd.iota(iota_row[:], [[1, 128]], base=0, channel_multiplier=0,
                       allow_small_or_imprecise_dtypes=True)
        pidx = const_pool.tile([128, 1], F32, tag="pidx")
        nc.gpsimd.iota(pidx[:], [[1, 1]], base=0, channel_multiplier=1,
                       allow_small_or_imprecise_dtypes=True)
        ident = const_pool.tile([128, 128], BF16, tag="ident")
        nc.vector.tensor_scalar(ident[:], iota_row[:], pidx[:], None,
                                AluOpType.is_equal)
        # rel_embed (fp8), fully resident; chunked load so dst-tile 0's
        # matmuls can start before the whole stream lands.  X rides the
        # scalar-engine HWDGE ring so it doesn't queue ahead of W's
        # sync-engine ring (per-engine FIFO).  Each dma_start costs its
        # sequencer ~0.67us, so the startup-critical X chunks are issued
        # first; pwt (needed only at the first projection, ~25us) after.
        x_sb = const_pool.tile([128, S_TILES, XDIM], FP8, tag="x")
        bounds = X_BOUNDS
        for i in range(len(bounds) - 1):
            lo, hi = bounds[i], bounds[i + 1]
            nc.scalar.dma_start(x_sb[:, lo:hi, :],
                                x_d[:, lo * XDIM:hi * XDIM])
        pwt_sb = const_pool.tile([128, 2, DIM], FP8, tag="pwt")
        nc.scalar.dma_start(pwt_sb[:], pwt_d[:])

        def ep_pre(t, agg_ps):
            # Stats + LN, no PE involvement.  var from E[x^2]-mu^2 so the
            # rowsum (DVE) and sum-of-squares (ACT) passes both read agg_ps
            # directly and run on different engines in parallel.
            rel_t = ep_pool.tile([128, DIM], F32, tag="rel")
            nc.scalar.dma_start(rel_t[:], relsl_d[t * 128:(t + 1) * 128, :])
            agg = ep_pool.tile([128, DIM], F32, tag="agg_sb")
            rowsum = ep_pool.tile([128, 1], F32, tag="rowsum")
            nc.vector.tensor_scalar(agg[:], agg_ps[:], 0.0, 0.0,
                                    AluOpType.add, AluOpType.add,
                                    accum_out=rowsum[:])
            sq = ep_pool.tile([128, DIM], BF16, tag="sq")
            sumsq = ep_pool.tile([128, 1], F32, tag="sumsq")
            nc.scalar.activation(sq[:], agg_ps[:], AF.Square,
                                 accum_out=sumsq[:])
            mean = ep_pool.tile([128, 1], F32, tag="mean")
            nc.scalar.mul(mean[:], rowsum[:], 1.0 / DIM)
            nmean = ep_pool.tile([128, 1], F32, tag="nmean")
            nc.vector.tensor_scalar(nmean[:], rowsum[:], -1.0 / DIM, None,
                                    AluOpType.mult)
            # bias_t = eps - mu^2
            bias_t = ep_pool.tile([128, 1], F32, tag="bias_t")
            nc.vector.tensor_scalar(bias_t[:], nmean[:], mean[:], LN_EPS,
                                    AluOpType.mult, AluOpType.add)
            std = ep_pool.tile([128, 1], F32, tag="std")
            nc.scalar.activation(std[:], sumsq[:], AF.Sqrt, bias=bias_t[:],
                                 scale=1.0 / DIM)
            rstd = ep_pool.tile([128, 1], F32, tag="rstd")
            nc.vector.reciprocal(rstd[:], std[:])
            ln = ep_pool.tile([128, DIM], BF16, tag="ln")
            nc.vector.tensor_scalar(ln[:], agg[:], mean[:], rstd[:],
                                    AluOpType.subtract, AluOpType.mult)
            return ln, rel_t

        def ep_tr(t, ln):
            # transpose ln (bf16) on the PE; both halves land in one PSUM
            # tile so a single ACT copy moves them out, folding in alpha
            tr_ps = ps_tr.tile([128, 2, 128], BF16, tag="tr")
            for k in range(2):
                nc.tensor.transpose(tr_ps[:, k, :],
                                    ln[:, k * 128:(k + 1) * 128], ident[:])
            lnT = ep_pool.tile([128, 2, 128], FP8, tag="lnT")
            nc.scalar.mul(lnT[:], tr_ps[:], ALPHA)
            return lnT

        def ep_y(t, lnT, rel_t, last=False):
            # one fp8 DoubleRow matmul does the whole 256-deep projection;
            # the residual add reads y straight from PSUM
            y_ps = ps_y.tile([128, DIM], F32, tag="y")
            nc.tensor.matmul(y_ps[:], lnT[:], pwt_sb[:],
                             start=True, stop=True,
                             perf_mode=mybir.MatmulPerfMode.DoubleRow)
            out_t = ep_pool.tile([128, DIM], BF16, tag="out")
            if last:
                # tail: stream the residual add + store in halves so the
                # final DMA starts half an add earlier
                for h in range(2):
                    sl = slice(h * 128, (h + 1) * 128)
                    nc.vector.tensor_tensor(out_t[:, sl], y_ps[:, sl],
                                            rel_t[:, sl], AluOpType.add)
                    nc.scalar.dma_start(out_d[t * 128:(t + 1) * 128, sl],
                                        out_t[:, sl])
            else:
                nc.vector.tensor_tensor(out_t[:], y_ps[:], rel_t[:],
                                        AluOpType.add)
                # scalar ring: keep the W (sync) FIFO free of stores
                nc.scalar.dma_start(out_d[t * 128:(t + 1) * 128, :],
                                    out_t[:])

        # software-pipelined at two depths: tile t-1's stats/LN + transposes
        # are emitted after tile t's MMs (LN chain hides under the MM run),
        # and tile t-1's projection matmul is emitted in the MIDDLE of tile
        # t+1's MMs, so the PE (strict program order) never stalls on the
        # ACT copy of lnT -- the copy hides under 20 more agg matmuls.
        prev = None   # (t, agg_ps): awaiting stats/LN/transpose
        ready = None  # (t, lnT, rel_t): awaiting projection + store
        for t in range(N_TILES):
            wb = (W_BOUNDS0 if t == 0
                  else W_BOUNDS1 if t == 1 else W_BOUNDS)
            w_t = wpool.tile([128, S_TILES, TILE], FP8, tag="w")
            for i in range(len(wb) - 1):
                lo, hi = wb[i], wb[i + 1]
                nc.sync.dma_start(w_t[:, lo:hi, :],
                                  w_d[t, :, lo * TILE:hi * TILE])

            agg_ps = ps_agg.tile([128, XDIM], F32, tag="agg")
            for j in range(S_PAIRS // 2):
                nc.tensor.matmul(agg_ps[:], w_t[:, 2 * j:2 * j + 2, :],
                                 x_sb[:, 2 * j:2 * j + 2, :],
                                 start=(j == 0), stop=False,
                                 perf_mode=mybir.MatmulPerfMode.DoubleRow)
            if ready is not None:
                ep_y(*ready)
                ready = None
            pend = None
            if t == N_TILES - 1 and prev is not None:
                # final iteration: pull tile 8's stats/LN chain forward to
                # the MID slot so it hides under the last 20 agg matmuls --
                # at the end-slot it has no matmul window left and the PE
                # would stall ~2us on its transposes (the old 3us tail gap)
                tq, pps = prev
                prev = None
                lnp, relp = ep_pre(tq, pps)
                pend = (tq, lnp, relp)
            for j in range(S_PAIRS // 2, S_PAIRS):
                nc.tensor.matmul(agg_ps[:], w_t[:, 2 * j:2 * j + 2, :],
                                 x_sb[:, 2 * j:2 * j + 2, :],
                                 start=False, stop=(j == S_PAIRS - 1),
                                 perf_mode=mybir.MatmulPerfMode.DoubleRow)
            if pend is not None:
                tq, lnp, relp = pend
                ready = (tq, ep_tr(tq, lnp), relp)
            elif prev is not None:
                tq, pps = prev
                ln, rel_t = ep_pre(tq, pps)
                ready = (tq, ep_tr(tq, ln), rel_t)
            prev = (t, agg_ps)
        # drain: enqueue tile 9's chain FIRST so it flows on DVE/ACT while
        # the PE runs tile 8's projection
        tp, pps = prev
        ln9, rel9 = ep_pre(tp, pps)
        ep_y(*ready)
        ep_y(tp, ep_tr(tp, ln9), rel9, last=True)

    nc.compile()
    return nc


def _prep(rel_embed, rel_edge_index, rel_edge_weight, proj_w):
    """Host-side sharding/layout: scatter edges into dense per-(dst tile,
    src tile) weight blocks; lay out rel_embed for SBUF residency."""
    src = np.asarray(rel_edge_index[0], dtype=np.int64)
    dst = np.asarray(rel_edge_index[1], dtype=np.int64)
    w = np.asarray(rel_edge_weight, dtype=np.float32)
    rel = np.asarray(rel_embed, dtype=np.float32)
    pw = np.asarray(proj_w, dtype=np.float32)

    core = dst // NODES_PER_CORE
    drel = dst - core * NODES_PER_CORE
    t = drel // TILE
    d = drel % TILE
    s = src // TILE
    p = src % TILE
    # flat index inside one core's [N_TILES, S_TILES, 128, 128] block array
    flat = ((t * S_TILES + s) * TILE + p) * TILE + d
    blk_sz = N_TILES * S_TILES * TILE * TILE

    w_dev = np.empty((N_CORES, N_TILES, 128, S_TILES * TILE),
                     dtype=ml_dtypes.float8_e4m3)
    for c in range(N_CORES):
        m = core == c
        wc = np.bincount(flat[m], weights=w[m], minlength=blk_sz)
        wc = wc.reshape(N_TILES, S_TILES, TILE, TILE).astype(np.float32)
        # -> [t, p(src), s*128+d(dst)] so the SBUF tile is partition=src
        w_dev[c] = wc.transpose(0, 2, 1, 3).reshape(
            N_TILES, 128, S_TILES * TILE)

    rel16 = rel.astype(ml_dtypes.float8_e4m3)
    rel16_pad = np.zeros((S_TILES * TILE, XDIM), dtype=ml_dtypes.float8_e4m3)
    rel16_pad[:N_NODES, :DIM] = rel16
    x_dev = np.ascontiguousarray(
        rel16_pad.reshape(S_TILES, TILE, XDIM).transpose(1, 0, 2).reshape(
            128, S_TILES * XDIM))

    relslice = np.zeros((N_CORES, OUT_ROWS, DIM), dtype=np.float32)
    for c in range(N_CORES):
        relslice[c, :NODES_PER_CORE] = rel[c * NODES_PER_CORE:
                                           (c + 1) * NODES_PER_CORE]
    pwt = pw.T.astype(ml_dtypes.float8_e4m3)  # [f, o]
    pwt_dev = np.ascontiguousarray(
        pwt.reshape(2, 128, DIM).transpose(1, 0, 2).reshape(128, 2 * DIM))

    in_maps = []
    for c in range(N_CORES):
        in_maps.append({
            "x": x_dev,
            "wblk": w_dev[c],
            "relslice": relslice[c],
            "projwT": pwt_dev,
        })
    return in_maps


def kernel(rel_embed, rel_edge_index, rel_edge_weight, proj_w,
           _trace=False):
    in_maps = _prep(rel_embed, rel_edge_index, rel_edge_weight, proj_w)
    nc = _CACHE.get("nc")
    if nc is None:
        nc = _build()
        _CACHE["nc"] = nc
    res = run_bass_kernel_spmd(nc, in_maps, core_ids=list(range(N_CORES)),
                               trace=_trace)
    out = np.concatenate(
        [res.results[c]["out"][:NODES_PER_CORE] for c in range(N_CORES)],
        axis=0)
    if _trace:
        kernel.last_results = res
    return out.astype(np.float32)



# revision 75
# speedup vs baseline: 1.0484x; 1.0371x over previous
"""DualRelGCN message-passing kernel for 8 TRN2 NeuronCores.

Strategy (destination-sharded, collective-free, block-dense):
  - LayerNorm is invariant to positive per-row scaling, so LN(agg/denom) ==
    LN(agg): the denominator drops out of the computation entirely.
  - Shard edges by dst range: core c owns nodes [1250c, 1250(c+1)) and
    receives every edge whose dst falls there.  Each core computes its 1250
    output rows completely locally -> no collectives.
  - The weighted gather+segment_sum is expressed as a block matmul:
    agg[tile t] = sum_s W_ts.T @ X_s, where W_ts is the [128 src, 128 dst]
    dense block of the weighted adjacency (host-scattered from the edge
    list; ~5% nnz but dense matmul on PE beats any descriptor-generated
    gather path by a wide margin) and X_s is a [128, 256] tile of rel_embed
    (bf16, fully resident in SBUF).  PSUM accumulates over s in fp32.
  - Epilogue per dst tile (critical-path optimized): LN stats via
    E[x^2]-mu^2 so the rowsum (DVE) and sum-of-squares (ACT) passes read
    the PSUM aggregate in parallel; LN is one fused (sub,mult) DVE op;
    alpha folds into the PSUM->SBUF transpose copy (single ACT op for both
    halves); the 256-deep projection is ONE fp8 DoubleRow matmul; the
    residual add reads y straight from PSUM.
  - Two-depth software pipeline: tile t-1's stats/LN + transposes are
    emitted after tile t's block matmuls, and tile t-1's projection matmul
    is emitted in the MIDDLE of tile t+1's matmul run, so the in-order PE
    queue never stalls on the ACT copy of lnT.
  - The output is stored bf16 (halves the final store; the host upcasts
    to f32) and the last tile's residual-add + store run in halves so the
    final DMA starts earlier.
  - The device program is fully static: the edge distribution only changes
    tensor *contents*, never the instruction stream.

Measured on the 8-core trn2 fixture: 78.8-79.6us HW exec in a quiet
machine phase (81-93us when the shared TRN2 is contended; identical
binaries swing by several us between phases) vs the 81.8us staged
baseline; rel fro error 5.4e-3.  The
kernel is delivery-bound: ~17MB/core of loads at ~360GB/s pins the
first half (DMA data flow starts ~10us after NEFF start regardless of
ring/issue order -- DGE bring-up floor; head ~13.6us until X+W0 land,
tile-1 W gap ~2us), then the PE stream (~51us busy, at the
fp8-DoubleRow column-rate floor: 1 output column/cycle means any dense
blocking costs >=102.4K columns = 44.5us) and a ~8us tail (last tile's
serial LN chain + store + teardown) close it out.  Things measured NOT
to help: earlier PE start via finer chunks (p-state stutter eats the
gain), PE p-state warm-up matmuls (ramped the clock perfectly but the
kernel is delivery-bound, so the PE just waits faster), single-ring
need-order DMA for X+W0 (PE start pinned at ~13.6 either way, and W1+
falls behind X's tail), W or out stores on other rings (a ring's
sequencer processes its DMA sem-waits in order, so one late-dependency
store blocks every load queued behind it), bf16 residual, deeper W
prefetch (pulls far-future W bytes into the startup-critical window).
"""

import sys

for _p in ("/opt/trn_rl_repo",):
    if _p not in sys.path:
        sys.path.insert(0, _p)

from contextlib import ExitStack

import numpy as np
import ml_dtypes

import concourse.bacc as bacc
import concourse.mybir as mybir
from concourse.alu_op_type import AluOpType
from concourse.tile import TileContext
from concourse.bass_utils import run_bass_kernel_spmd

F32 = mybir.dt.float32
BF16 = mybir.dt.bfloat16
FP8 = mybir.dt.float8e4
AF = mybir.ActivationFunctionType

N_NODES = 10000
DIM = 256
N_CORES = 8
NODES_PER_CORE = N_NODES // N_CORES  # 1250
TILE = 128
N_TILES = -(-NODES_PER_CORE // TILE)  # 10 dst tiles per core
S_TILES = -(-N_NODES // TILE) + 1  # 79 src tiles + 1 zero pad -> even count
S_PAIRS = S_TILES // 2  # DoubleRow matmuls contract two src tiles at once
OUT_ROWS = N_TILES * TILE  # 1280
ALPHA = 0.1
LN_EPS = 1e-5
# chunk boundaries (src-tile units): small leading chunks let the first
# matmuls start before the bulk of X/W lands
X_BOUNDS = [0, 20, 40, 60, 80]
XDIM = DIM
W_BOUNDS0 = [0, 20, 40, 60, 80]  # dst tile 0 (startup critical)
W_BOUNDS1 = [0, 8, 20, 40, 60, 80]  # tile 1: PE restarts off a small chunk
W_BOUNDS = [0, 20, 40, 60, 80]  # steady-state tiles

_CACHE: dict = {}


def _build():
    nc = bacc.Bacc("TRN2", target_bir_lowering=False, debug=False,
                   num_devices=N_CORES)

    x_d = nc.dram_tensor("x", [128, S_TILES * XDIM], FP8,
                        kind="ExternalInput")
    w_d = nc.dram_tensor("wblk", [N_TILES, 128, S_TILES * TILE], FP8,
                         kind="ExternalInput")
    relsl_d = nc.dram_tensor("relslice", [OUT_ROWS, DIM], F32,
                             kind="ExternalInput")
    pwt_d = nc.dram_tensor("projwT", [128, 2 * DIM], FP8,
                           kind="ExternalInput")
    out_d = nc.dram_tensor("out", [OUT_ROWS, DIM], BF16,
                           kind="ExternalOutput")

    with TileContext(nc) as tc, ExitStack() as es:
        const_pool = es.enter_context(tc.tile_pool(name="const", bufs=1))
        wpool = es.enter_context(tc.tile_pool(name="wblk", bufs=4))
        ep_pool = es.enter_context(tc.tile_pool(name="ep", bufs=3))
        ps_agg = es.enter_context(tc.tile_pool(name="ps_agg", bufs=3,
                                               space="PSUM"))
        ps_tr = es.enter_context(tc.tile_pool(name="ps_tr", bufs=2,
                                              space="PSUM"))
        ps_y = es.enter_context(tc.tile_pool(name="ps_y", bufs=3,
                                             space="PSUM"))


        # --- constants / resident inputs ---
        iota_row = const_pool.tile([128, 128], F32, tag="iota")
        nc.gpsimd.iota(iota_row[:], [[1, 128]], base=0, channel_multiplier=0,
                       allow_small_or_imprecise_dtypes=True)
        pidx = const_pool.tile([128, 1], F32, tag="pidx")
        nc.gpsimd.iota(pidx[:], [[1, 1]], base=0, channel_multiplier=1,
                       allow_small_or_imprecise_dtypes=True)
        ident = const_pool.tile([128, 128], BF16, tag="ident")
        nc.vector.tensor_scalar(ident[:], iota_row[:], pidx[:], None,
                                AluOpType.is_equal)
        # rel_embed (fp8), fully resident; chunked load so dst-tile 0's
        # matmuls can start before the whole stream lands.  X rides the
        # scalar-engine HWDGE ring so it doesn't queue ahead of W's
        # sync-engine ring (per-engine FIFO).  Each dma_start costs its
        # sequencer ~0.67us, so the startup-critical X chunks are issued
        # first; pwt (needed only at the first projection, ~25us) after.
        x_sb = const_pool.tile([128, S_TILES, XDIM], FP8, tag="x")
        bounds = X_BOUNDS
        for i in range(len(bounds) - 1):
            lo, hi = bounds[i], bounds[i + 1]
            nc.scalar.dma_start(x_sb[:, lo:hi, :],
                                x_d[:, lo * XDIM:hi * XDIM])
        pwt_sb = const_pool.tile([128, 2, DIM], FP8, tag="pwt")
        nc.scalar.dma_start(pwt_sb[:], pwt_d[:])

        def ep_pre(t, agg_ps):
            # Stats + LN, no PE involvement.  var from E[x^2]-mu^2 so the
            # rowsum (DVE) and sum-of-squares (ACT) passes both read agg_ps
            # directly and run on different engines in parallel.
            rel_t = ep_pool.tile([128, DIM], F32, tag="rel")
            nc.scalar.dma_start(rel_t[:], relsl_d[t * 128:(t + 1) * 128, :])
            agg = ep_pool.tile([128, DIM], F32, tag="agg_sb")
            rowsum = ep_pool.tile([128, 1], F32, tag="rowsum")
            nc.vector.tensor_scalar(agg[:], agg_ps[:], 0.0, 0.0,
                                    AluOpType.add, AluOpType.add,
                                    accum_out=rowsum[:])
            sq = ep_pool.tile([128, DIM], BF16, tag="sq")
            sumsq = ep_pool.tile([128, 1], F32, tag="sumsq")
            nc.scalar.activation(sq[:], agg_ps[:], AF.Square,
                                 accum_out=sumsq[:])
            mean = ep_pool.tile([128, 1], F32, tag="mean")
            nc.vector.tensor_scalar(mean[:], rowsum[:], 1.0 / DIM, None,
                                    AluOpType.mult)
            nmean = ep_pool.tile([128, 1], F32, tag="nmean")
            nc.vector.tensor_scalar(nmean[:], rowsum[:], -1.0 / DIM, None,
                                    AluOpType.mult)
            # bias_t = eps - mu^2
            bias_t = ep_pool.tile([128, 1], F32, tag="bias_t")
            nc.vector.tensor_scalar(bias_t[:], nmean[:], mean[:], LN_EPS,
                                    AluOpType.mult, AluOpType.add)
            std = ep_pool.tile([128, 1], F32, tag="std")
            nc.scalar.activation(std[:], sumsq[:], AF.Sqrt, bias=bias_t[:],
                                 scale=1.0 / DIM)
            rstd = ep_pool.tile([128, 1], F32, tag="rstd")
            nc.vector.reciprocal(rstd[:], std[:])
            ln = ep_pool.tile([128, DIM], BF16, tag="ln")
            nc.vector.tensor_scalar(ln[:], agg[:], mean[:], rstd[:],
                                    AluOpType.subtract, AluOpType.mult)
            return ln, rel_t

        def ep_tr(t, ln):
            # transpose ln (bf16) on the PE; both halves land in one PSUM
            # tile so a single ACT copy moves them out, folding in alpha
            tr_ps = ps_tr.tile([128, 2, 128], BF16, tag="tr")
            for k in range(2):
                nc.tensor.transpose(tr_ps[:, k, :],
                                    ln[:, k * 128:(k + 1) * 128], ident[:])
            lnT = ep_pool.tile([128, 2, 128], FP8, tag="lnT")
            nc.scalar.mul(lnT[:], tr_ps[:], ALPHA)
            return lnT

        def ep_y(t, lnT, rel_t, last=False):
            # one fp8 DoubleRow matmul does the whole 256-deep projection;
            # the residual add reads y straight from PSUM
            y_ps = ps_y.tile([128, DIM], F32, tag="y")
            nc.tensor.matmul(y_ps[:], lnT[:], pwt_sb[:],
                             start=True, stop=True,
                             perf_mode=mybir.MatmulPerfMode.DoubleRow)
            out_t = ep_pool.tile([128, DIM], BF16, tag="out")
            if last:
                # tail: stream the residual add + store in halves so the
                # final DMA starts half an add earlier
                for h in range(2):
                    sl = slice(h * 128, (h + 1) * 128)
                    nc.vector.tensor_tensor(out_t[:, sl], y_ps[:, sl],
                                            rel_t[:, sl], AluOpType.add)
                    nc.scalar.dma_start(out_d[t * 128:(t + 1) * 128, sl],
                                        out_t[:, sl])
            else:
                nc.vector.tensor_tensor(out_t[:], y_ps[:], rel_t[:],
                                        AluOpType.add)
                # scalar ring: keep the W (sync) FIFO free of stores
                nc.scalar.dma_start(out_d[t * 128:(t + 1) * 128, :],
                                    out_t[:])

        # software-pipelined at two depths: tile t-1's stats/LN + transposes
        # are emitted after tile t's MMs (LN chain hides under the MM run),
        # and tile t-1's projection matmul is emitted in the MIDDLE of tile
        # t+1's MMs, so the PE (strict program order) never stalls on the
        # ACT copy of lnT -- the copy hides under 20 more agg matmuls.
        prev = None   # (t, agg_ps): awaiting stats/LN/transpose
        ready = None  # (t, lnT, rel_t): awaiting projection + store
        for t in range(N_TILES):
            wb = (W_BOUNDS0 if t == 0
                  else W_BOUNDS1 if t == 1 else W_BOUNDS)
            w_t = wpool.tile([128, S_TILES, TILE], FP8, tag="w")
            for i in range(len(wb) - 1):
                lo, hi = wb[i], wb[i + 1]
                nc.sync.dma_start(w_t[:, lo:hi, :],
                                  w_d[t, :, lo * TILE:hi * TILE])

            agg_ps = ps_agg.tile([128, XDIM], F32, tag="agg")
            for j in range(S_PAIRS // 2):
                nc.tensor.matmul(agg_ps[:], w_t[:, 2 * j:2 * j + 2, :],
                                 x_sb[:, 2 * j:2 * j + 2, :],
                                 start=(j == 0), stop=False,
                                 perf_mode=mybir.MatmulPerfMode.DoubleRow)
            if ready is not None:
                ep_y(*ready)
                ready = None
            pend = None
            if t == N_TILES - 1 and prev is not None:
                # final iteration: pull tile 8's stats/LN chain forward to
                # the MID slot so it hides under the last 20 agg matmuls --
                # at the end-slot it has no matmul window left and the PE
                # would stall ~2us on its transposes (the old 3us tail gap)
                tq, pps = prev
                prev = None
                lnp, relp = ep_pre(tq, pps)
                pend = (tq, lnp, relp)
            for j in range(S_PAIRS // 2, S_PAIRS):
                nc.tensor.matmul(agg_ps[:], w_t[:, 2 * j:2 * j + 2, :],
                                 x_sb[:, 2 * j:2 * j + 2, :],
                                 start=False, stop=(j == S_PAIRS - 1),
                                 perf_mode=mybir.MatmulPerfMode.DoubleRow)
            if pend is not None:
                tq, lnp, relp = pend
                ready = (tq, ep_tr(tq, lnp), relp)
            elif prev is not None:
                tq, pps = prev
                ln, rel_t = ep_pre(tq, pps)
                ready = (tq, ep_tr(tq, ln), rel_t)
            prev = (t, agg_ps)
        # drain: enqueue tile 9's chain FIRST so it flows on DVE/ACT while
        # the PE runs tile 8's projection
        tp, pps = prev
        ln9, rel9 = ep_pre(tp, pps)
        ep_y(*ready)
        ep_y(tp, ep_tr(tp, ln9), rel9, last=True)

    nc.compile()
    return nc


def _prep(rel_embed, rel_edge_index, rel_edge_weight, proj_w):
    """Host-side sharding/layout: scatter edges into dense per-(dst tile,
    src tile) weight blocks; lay out rel_embed for SBUF residency."""
    src = np.asarray(rel_edge_index[0], dtype=np.int64)
    dst = np.asarray(rel_edge_index[1], dtype=np.int64)
    w = np.asarray(rel_edge_weight, dtype=np.float32)
    rel = np.asarray(rel_embed, dtype=np.float32)
    pw = np.asarray(proj_w, dtype=np.float32)

    core = dst // NODES_PER_CORE
    drel = dst - core * NODES_PER_CORE
    t = drel // TILE
    d = drel % TILE
    s = src // TILE
    p = src % TILE
    # flat index inside one core's [N_TILES, S_TILES, 128, 128] block array
    flat = ((t * S_TILES + s) * TILE + p) * TILE + d
    blk_sz = N_TILES * S_TILES * TILE * TILE

    w_dev = np.empty((N_CORES, N_TILES, 128, S_TILES * TILE),
                     dtype=ml_dtypes.float8_e4m3)
    for c in range(N_CORES):
        m = core == c
        wc = np.bincount(flat[m], weights=w[m], minlength=blk_sz)
        wc = wc.reshape(N_TILES, S_TILES, TILE, TILE).astype(np.float32)
        # -> [t, p(src), s*128+d(dst)] so the SBUF tile is partition=src
        w_dev[c] = wc.transpose(0, 2, 1, 3).reshape(
            N_TILES, 128, S_TILES * TILE)

    rel16 = rel.astype(ml_dtypes.float8_e4m3)
    rel16_pad = np.zeros((S_TILES * TILE, XDIM), dtype=ml_dtypes.float8_e4m3)
    rel16_pad[:N_NODES, :DIM] = rel16
    x_dev = np.ascontiguousarray(
        rel16_pad.reshape(S_TILES, TILE, XDIM).transpose(1, 0, 2).reshape(
            128, S_TILES * XDIM))

    relslice = np.zeros((N_CORES, OUT_ROWS, DIM), dtype=np.float32)
    for c in range(N_CORES):
        relslice[c, :NODES_PER_CORE] = rel[c * NODES_PER_CORE:
                                           (c + 1) * NODES_PER_CORE]
    pwt = pw.T.astype(ml_dtypes.float8_e4m3)  # [f, o]
    pwt_dev = np.ascontiguousarray(
        pwt.reshape(2, 128, DIM).transpose(1, 0, 2).reshape(128, 2 * DIM))

    in_maps = []
    for c in range(N_CORES):
        in_maps.append({
            "x": x_dev,
            "wblk": w_dev[c],
            "relslice": relslice[c],
            "projwT": pwt_dev,
        })
    return in_maps


def kernel(rel_embed, rel_edge_index, rel_edge_weight, proj_w,
           _trace=False):
    in_maps = _prep(rel_embed, rel_edge_index, rel_edge_weight, proj_w)
    nc = _CACHE.get("nc")
    if nc is None:
        nc = _build()
        _CACHE["nc"] = nc
    res = run_bass_kernel_spmd(nc, in_maps, core_ids=list(range(N_CORES)),
                               trace=_trace)
    out = np.concatenate(
        [res.results[c]["out"][:NODES_PER_CORE] for c in range(N_CORES)],
        axis=0)
    if _trace:
        kernel.last_results = res
    return out.astype(np.float32)

